# revision 42
# baseline (speedup 1.0000x reference)
"""Multi-head causal attention (B=2, S=2048, HID=2048, H=16, D=128) on 8 TRN2
NeuronCores.

Sharding: core c handles batch b=c//4 and heads [4*(c%4) .. 4*(c%4)+3].
Each core computes qkv-projection + RoPE + causal attention + its partial
out-projection; the host sums the 4 partial outputs per batch (tensor-parallel
reduce) and stacks the 2 batches.

On-chip layout: all activations are kept transposed ([feature, token]) so the
whole chain runs on the PE array with no on-device transposes:
  qT/kT = W_qk^T-slice @ x^T   (RoPE applied during PSUM evacuation)
  S^T[k,q] = kT^T@qT ; A = exp(S^T*scale) (*causal mask)
  outT[d,q] = V^T-chunks @ A   (accumulated over k chunks)
  y[tok,col] = outT^T-chunks @ W_o-rows  (accumulated over heads)
Softmax row-sums come from a ones[128,128] matmul in the [k,q] layout (the
output is the denominator already broadcast across partitions); exp runs on
paired k-chunks ([128,1024] tiles) to amortize ACT overhead.

All matmul operands are bf16 (same 1 cycle/row PE rate as f32r at free>=256,
half the DMA and SBUF), weights and x are DMA'd exactly once (weights stay
resident; the V and QK projections share each x token-block), softmax
reciprocal runs on ACT instead of DVE, and phase-3 results DMA straight from
PSUM to DRAM.
"""
import sys

sys.path.insert(0, '/opt/trn_rl_repo')

import numpy as np

B, S, HID = 2, 2048, 2048
H, D = 16, 128
NH = H // 4          # heads per core = 4
HC = HID // 128      # hid chunks = 16
TB = 512             # token block for projection
NTB = S // TB        # 4
QB = 512             # q block in attention
NQB = S // QB        # 4
NKCH = S // 128      # k chunks total = 16
SCALE = 1.0 / float(np.sqrt(D))
BASE = 10000.0
N_CORES = 8

_cache = {}


def _build():
    import concourse.bass as bass  # noqa: F401
    import concourse.tile as tile
    from concourse import bacc, mybir

    f32 = mybir.dt.float32
    bf16 = mybir.dt.bfloat16
    fp8 = mybir.dt.float8e4
    DR = mybir.MatmulPerfMode.DoubleRow
    EXP = mybir.ActivationFunctionType.Exp
    MULT = mybir.AluOpType.mult
    ADD = mybir.AluOpType.add
    # constant subtracted from scaled scores before exp so e^s fits fp8-e4m3
    # range (max causal score ~6.5 -> e^5.0 = 148 < 448); cancels in softmax
    CBIAS = -1.5

    nc = bacc.Bacc("TRN2", target_bir_lowering=False, debug=False,
                   num_devices=N_CORES)

    # x and the qkv weights arrive pre-tiled in their exact SBUF layouts so
    # every load is one contiguous large-line DMA
    xT = nc.dram_tensor("xT", [128, NTB * HC * TB], bf16,
                        kind="ExternalInput").ap()
    wqk = nc.dram_tensor("wqk", [128, 8 * HC * 128], bf16,
                         kind="ExternalInput").ap()
    wv = nc.dram_tensor("wv", [128, HC * NH * D], bf16,
                        kind="ExternalInput").ap()
    wo = nc.dram_tensor("wo", [NH * D, HID], bf16, kind="ExternalInput").ap()
    cosT = nc.dram_tensor("cosT", [D, S], bf16, kind="ExternalInput").ap()
    sinS = nc.dram_tensor("sinS", [D, S], bf16, kind="ExternalInput").ap()
    maskT = nc.dram_tensor("maskT", [128, 4 * QB], bf16, kind="ExternalInput").ap()
    ones_sq = nc.dram_tensor("ones_sq", [128, 128], bf16, kind="ExternalInput").ap()
    ones_8 = nc.dram_tensor("ones_8", [128, 256], fp8, kind="ExternalInput").ap()
    cbias = nc.dram_tensor("cbias", [128, 1], f32, kind="ExternalInput").ap()
    y = nc.dram_tensor("y", [S, HID], bf16, kind="ExternalOutput").ap()

    with tile.TileContext(nc) as tc:
        with tc.tile_pool(name="persist", bufs=1) as pp:
            # resident weights / tables (issued in dependency-urgency order:
            # wvt + first x block unblock the first matmul chain)
            wvt = pp.tile([128, HC * NH * D], bf16, tag="wvt")
            wqkt = pp.tile([128, 8 * HC * 128], bf16, tag="wqkt")
            cosF = pp.tile([D, S], bf16, tag="cosF")
            sinF = pp.tile([D, S], bf16, tag="sinF")
            tmask = pp.tile([128, 4 * QB], bf16, tag="tmask")
            t1s = pp.tile([128, 128], bf16, tag="t1s")
            t1s8 = pp.tile([128, 256], fp8, tag="t1s8")
            tcb = pp.tile([128, 1], f32, tag="tcb")
            wot = [pp.tile([128, HID], bf16, tag=f"wot{h}", name=f"wot{h}")
                   for h in range(NH)]

            # persistent activations
            qkT = [pp.tile([128, S], bf16, tag=f"qkT{i}", name=f"qkT{i}")
                   for i in range(8)]
            v_all = pp.tile([128, NKCH * NH * D], bf16, tag="v_all")
            outT = [pp.tile([128, S], bf16, tag=f"outT{h}", name=f"outT{h}")
                    for h in range(NH)]

            # ---- phase 1: V + Q/K projections, x streamed once ----
            with tc.tile_pool(name="p1x", bufs=2) as p1x, \
                 tc.tile_pool(name="rope", bufs=2) as rp, \
                 tc.tile_pool(name="psV", bufs=1, space="PSUM") as psV, \
                 tc.tile_pool(name="psQK", bufs=3, space="PSUM") as psQK:

                def load_xtb(jb):
                    # one large DMA: big transfers shard across all 16 DMA
                    # engines (~360GB/s); small per-chunk DMAs do not
                    xTb = p1x.tile([128, HC * TB], bf16, tag="xTb")
                    nc.sync.dma_start(
                        xTb[:], xT[:, jb * HC * TB:(jb + 1) * HC * TB])
                    return xTb

                # loads in strict consumption order; wvt and the first x
                # block are quarter-split and interleaved so the c-outer V
                # chain starts after ~512KB and then follows the DMA stream
                xTb0 = p1x.tile([128, HC * TB], bf16, tag="xTb")
                WQ = HC * NH * D // 4
                XQ = HC * TB // 4
                for q in range(4):
                    nc.sync.dma_start(wvt[:, q * WQ:(q + 1) * WQ],
                                      wv[:, q * WQ:(q + 1) * WQ])
                    nc.sync.dma_start(xTb0[:, q * XQ:(q + 1) * XQ],
                                      xT[:, q * XQ:(q + 1) * XQ])
                xtiles = [xTb0]
                nc.sync.dma_start(wqkt[:, :4 * HC * 128],
                                  wqk[:, :4 * HC * 128])
                nc.sync.dma_start(wqkt[:, 4 * HC * 128:],
                                  wqk[:, 4 * HC * 128:])
                nc.sync.dma_start(cosF[:, 0:TB], cosT[:, 0:TB])
                nc.sync.dma_start(sinF[:, 0:TB], sinS[:, 0:TB])
                xtiles.append(load_xtb(1))
                nc.sync.dma_start(cosF[:, TB:], cosT[:, TB:])
                nc.sync.dma_start(sinF[:, TB:], sinS[:, TB:])
                nc.sync.dma_start(tmask[:], maskT[:])
                nc.sync.dma_start(t1s[:], ones_sq[:])
                nc.sync.dma_start(t1s8[:], ones_8[:])
                nc.sync.dma_start(tcb[:], cbias[:])
                for h in range(NH):
                    nc.sync.dma_start(wot[h][:], wo[h * 128:(h + 1) * 128, :])

                for jb in range(NTB):
                    xTb = xtiles[jb]
                    # V projection (c-outer: follows the chunk DMA stream):
                    # 4 accumulators [128,512] in one 4-bank PSUM tile,
                    # evacuated with a single wide copy
                    Pv = psV.tile([128, 4 * NH * D], f32, tag="Pv")
                    for c in range(HC):
                        for t2 in range(TB // 128):
                            nc.tensor.matmul(
                                Pv[:, t2 * NH * D:(t2 + 1) * NH * D],
                                xTb[:, c * TB + t2 * 128:
                                    c * TB + (t2 + 1) * 128],
                                wvt[:, c * NH * D:(c + 1) * NH * D],
                                start=(c == 0), stop=(c == HC - 1))
                    nc.scalar.copy(
                        v_all[:, jb * 4 * NH * D:(jb + 1) * 4 * NH * D],
                        Pv[:])
                    # Q/K projection + RoPE
                    sl = slice(jb * TB, (jb + 1) * TB)
                    for cc in range(8):  # 4 q cols then 4 k cols
                        P = psQK.tile([128, TB], f32, tag="P")
                        for c in range(HC):
                            nc.tensor.matmul(
                                P[:],
                                wqkt[:, cc * HC * 128 + c * 128:
                                     cc * HC * 128 + (c + 1) * 128],
                                xTb[:, c * TB:(c + 1) * TB],
                                start=(c == 0), stop=(c == HC - 1))
                        u = rp.tile([128, TB], f32, tag="u")
                        nc.vector.tensor_copy(u[:], P[:])
                        rot = rp.tile([128, TB], f32, tag="rot")
                        nc.sync.dma_start(rot[0:64, :], u[64:128, :])
                        nc.sync.dma_start(rot[64:128, :], u[0:64, :])
                        m = rp.tile([128, TB], f32, tag="m")
                        nc.vector.tensor_tensor(
                            out=m[:], in0=rot[:], in1=sinF[:, sl], op=MULT)
                        t = rp.tile([128, TB], f32, tag="t")
                        nc.vector.tensor_tensor(
                            out=t[:], in0=u[:], in1=cosF[:, sl], op=MULT)
                        nc.vector.tensor_tensor(
                            out=qkT[cc][:, sl], in0=t[:], in1=m[:], op=ADD)
                    # prefetch x for jb+2 (after this jb's compute is issued
                    # so the buffer-rotation WAR dependency is correct)
                    if jb + 2 < NTB:
                        xtiles.append(load_xtb(jb + 2))

            # ---- phase 2: attention ----
            with tc.tile_pool(name="p2", bufs=4) as p2, \
                 tc.tile_pool(name="p2r", bufs=2) as p2r, \
                 tc.tile_pool(name="psS", bufs=2, space="PSUM") as psS, \
                 tc.tile_pool(name="psO", bufs=2, space="PSUM") as psO, \
                 tc.tile_pool(name="psR", bufs=2, space="PSUM") as psR:
                # globally software-pipelined (depth 2) across (jb4, h)
                # blocks: the O/R matmuls for pair k issue during pair k+2's
                # slot so the ACT exp (+ DVE mask) latency never stalls the
                # PE. Diagonal chunks at relative offset md only need
                # q >= 128*md — S/exp/mask/O/R all run on the narrowed range.
                def flush(p):
                    (kc0, A, O, R, nkc, h, qlos, last, qsl, is8) = p
                    for i in range(2):
                        kc = kc0 + i
                        ql = qlos[i]
                        nc.tensor.matmul(
                            O[:, ql:QB],
                            v_all[:, kc * NH * D + h * D:
                                  kc * NH * D + (h + 1) * D],
                            A[:, i * QB + ql:(i + 1) * QB],
                            start=(kc == 0), stop=(kc == nkc - 1),
                            skip_group_check=True)
                        if not is8:
                            nc.tensor.matmul(
                                R[:, ql:QB], t1s[:],
                                A[:, i * QB + ql:(i + 1) * QB],
                                start=(kc == 0), stop=(kc == nkc - 1),
                                skip_group_check=True)
                    if is8:
                        # one DoubleRow matmul sums both fp8 chunks at once
                        nc.tensor.matmul(
                            R[:],
                            t1s8[:].rearrange("p (two n) -> p two n", two=2),
                            A[:].rearrange("p (two q) -> p two q", two=2),
                            start=(kc0 == 0), stop=(kc0 + 1 == nkc - 1),
                            perf_mode=DR, skip_group_check=True)
                    if last:
                        rec = p2r.tile([128, QB], f32, tag="rec")
                        nc.vector.reciprocal_approx_fast(rec[:], R[:])
                        nc.vector.tensor_tensor(
                            out=outT[h][:, qsl], in0=O[:], in1=rec[:],
                            op=MULT)

                pending = []
                for jb4 in range(NQB):
                    qsl = slice(jb4 * QB, (jb4 + 1) * QB)
                    for h in range(NH):
                        qT_h, kT_h = qkT[h], qkT[NH + h]
                        O = psO.tile([128, QB], f32, tag="O")
                        R = psR.tile([128, QB], f32, tag="R")
                        nkc = (QB // 128) * (jb4 + 1)
                        for kp in range(nkc // 2):  # paired k-chunks
                            kc0 = 2 * kp
                            md = kc0 - (QB // 128) * jb4
                            # per-half diagonal offset and narrowed q range
                            mds = [kc0 + i - (QB // 128) * jb4
                                   for i in range(2)]
                            qlos = [max(0, 128 * m) for m in mds]
                            Sc = psS.tile([128, 2 * QB], f32, tag="S")
                            for i in range(2):
                                ql = qlos[i]
                                nc.tensor.matmul(
                                    Sc[:, i * QB + ql:(i + 1) * QB],
                                    kT_h[:, (kc0 + i) * 128:(kc0 + i + 1) * 128],
                                    qT_h[:, jb4 * QB + ql:(jb4 + 1) * QB],
                                    start=True, stop=True)
                            if md >= 0:  # diagonal pair: exp+mask per half
                                A = p2.tile([128, 2 * QB], bf16, tag="A",
                                            bufs=4)
                                Araw = p2.tile([128, 2 * QB], bf16, tag="Araw",
                                               bufs=4)
                                for i in range(2):
                                    ql = qlos[i]
                                    sl_i = slice(i * QB + ql, (i + 1) * QB)
                                    nc.scalar.activation(
                                        Araw[:, sl_i], Sc[:, sl_i], EXP,
                                        bias=tcb[:], scale=SCALE)
                                    nc.vector.tensor_tensor(
                                        out=A[:, sl_i], in0=Araw[:, sl_i],
                                        in1=tmask[:, mds[i] * QB + ql:
                                                  (mds[i] + 1) * QB],
                                        op=MULT)
                                is8 = False
                            else:
                                A = p2.tile([128, 2 * QB], fp8, tag="A8",
                                            bufs=4)
                                nc.scalar.activation(A[:], Sc[:], EXP,
                                                     bias=tcb[:], scale=SCALE)
                                is8 = True
                            if len(pending) >= 2:
                                flush(pending.pop(0))
                            pending.append((kc0, A, O, R, nkc, h, qlos,
                                            kp == nkc // 2 - 1, qsl, is8))
                for p in pending:
                    flush(p)

            # ---- phase 3: out projection (partial) ----
            with tc.tile_pool(name="p3", bufs=4) as p3, \
                 tc.tile_pool(name="ps3", bufs=4, space="PSUM") as ps3:
                for tch in range(S // 128):
                    for cb in range(HID // 512):
                        P3 = ps3.tile([128, 512], f32, tag="P3")
                        for h in range(NH):
                            nc.tensor.matmul(
                                P3[:],
                                outT[h][:, tch * 128:(tch + 1) * 128],
                                wot[h][:, cb * 512:(cb + 1) * 512],
                                start=(h == 0), stop=(h == NH - 1))
                        ys = p3.tile([128, 512], bf16, tag="ys")
                        if (tch * 4 + cb) % 2 == 0:
                            nc.vector.tensor_copy(ys[:], P3[:])
                        else:
                            nc.scalar.copy(ys[:], P3[:])
                        nc.sync.dma_start(
                            y[tch * 128:(tch + 1) * 128,
                              cb * 512:(cb + 1) * 512], ys[:])

    nc.compile()
    return nc


def _host_inputs(x, w_qkv, w_out):
    """Build the 8 per-core input maps."""
    import ml_dtypes
    bf16 = ml_dtypes.bfloat16

    # RoPE tables, transposed ([d, t]) with the rotate-half sign folded in.
    inv_freq = 1.0 / (BASE ** (np.arange(0, D, 2, dtype=np.float64) / D))
    pos = np.arange(S, dtype=np.float64)
    freqs = np.outer(inv_freq, pos)           # [64, S]
    cos_h = np.cos(freqs).astype(np.float32)
    sin_h = np.sin(freqs).astype(np.float32)
    cosT = np.concatenate([cos_h, cos_h], 0).astype(bf16)   # [128, S]
    sinS = np.concatenate([-sin_h, sin_h], 0).astype(bf16)  # signed sin

    # Causal masks for the 4 diagonal sub-blocks ([k-part, q-free])
    kp = np.arange(128)[:, None]
    qf = np.arange(QB)[None, :]
    maskT = np.concatenate(
        [(qf >= 128 * mm + kp).astype(bf16) for mm in range(4)], axis=1)

    w3 = np.asarray(w_qkv, np.float32).reshape(HID, 3, H, D)
    wo_full = np.asarray(w_out, np.float32).reshape(H, D, HID)
    x = np.asarray(x, np.float32)

    shared = {
        "cosT": cosT, "sinS": sinS, "maskT": maskT,
        "ones_sq": np.ones((128, 128), bf16),
        "ones_8": np.ones((128, 256), ml_dtypes.float8_e4m3fn),
        "cbias": np.full((128, 1), -1.5, np.float32),
    }
    # x pre-tiled to the SBUF layout: [p, jb, c, t]
    xt_b = []
    for b in range(B):
        xt = x[b].T.reshape(HC, 128, NTB, TB).transpose(1, 2, 0, 3)
        xt_b.append(np.ascontiguousarray(xt.reshape(128, -1)).astype(bf16))

    in_maps = []
    for c in range(N_CORES):
        b, hg = c // 4, c % 4
        heads = slice(4 * hg, 4 * hg + 4)
        wqk_c = w3[:, 0:2, heads, :].reshape(HID, 2 * NH * D)
        wqk_c = wqk_c.reshape(HC, 128, 8, 128).transpose(1, 2, 0, 3)
        wqk_c = np.ascontiguousarray(wqk_c.reshape(128, -1)).astype(bf16)
        wv_c = w3[:, 2, heads, :].reshape(HID, NH * D)
        wv_c = wv_c.reshape(HC, 128, NH * D).transpose(1, 0, 2)
        wv_c = np.ascontiguousarray(wv_c.reshape(128, -1)).astype(bf16)
        wo_c = np.ascontiguousarray(
            wo_full[heads].reshape(NH * D, HID)).astype(bf16)
        in_maps.append({
            "xT": xt_b[b], "wqk": wqk_c, "wv": wv_c, "wo": wo_c, **shared,
        })
    return in_maps


def kernel(x, w_qkv, w_out):
    from concourse.bass_utils import run_bass_kernel_spmd

    if "nc" not in _cache:
        _cache["nc"] = _build()
    nc = _cache["nc"]
    in_maps = _host_inputs(x, w_qkv, w_out)
    res = run_bass_kernel_spmd(nc, in_maps, core_ids=list(range(N_CORES)))
    out = np.zeros((B, S, HID), np.float32)
    for c in range(N_CORES):
        out[c // 4] += res.results[c]["y"].astype(np.float32)
    return out


# revision 43
# speedup vs baseline: 1.0041x; 1.0041x over previous
"""Multi-head causal attention (B=2, S=2048, HID=2048, H=16, D=128) on 8 TRN2
NeuronCores.

Sharding: core c handles batch b=c//4 and heads [4*(c%4) .. 4*(c%4)+3].
Each core computes qkv-projection + RoPE + causal attention + its partial
out-projection; the host sums the 4 partial outputs per batch (tensor-parallel
reduce) and stacks the 2 batches.

On-chip layout: all activations are kept transposed ([feature, token]) so the
whole chain runs on the PE array with no on-device transposes:
  qT/kT = W_qk^T-slice @ x^T   (RoPE applied during PSUM evacuation)
  S^T[k,q] = kT^T@qT ; A = exp(S^T*scale) (*causal mask)
  outT[d,q] = V^T-chunks @ A   (accumulated over k chunks)
  y[tok,col] = outT^T-chunks @ W_o-rows  (accumulated over heads)
Softmax row-sums come from a ones[128,128] matmul in the [k,q] layout (the
output is the denominator already broadcast across partitions); exp runs on
paired k-chunks ([128,1024] tiles) to amortize ACT overhead.

Performance structure (~330us on 8 cores, ~2.4GHz PE cadence):
- all matmul operands bf16 (1 cycle/row, half the DMA/SBUF of f32r); weights
  and x live in SBUF and are DMA'd exactly once as large contiguous
  transfers (big DMAs shard across all 16 engines; x/wvt are quarter-split
  and interleaved so the c-outer V chain starts after ~512KB)
- phase 2 is globally software-pipelined (depth 2) so ACT exp latency never
  stalls the PE; diagonal chunks are narrowed to q >= 128*md in S/exp/mask/
  O/R; off-diagonal pairs write exp output as fp8-e4m3 (scores get a -1.5
  bias pre-exp, cancelled by the softmax ratio, to fit e4m3 range) so the
  row-sum matmul runs one DoubleRow matmul per chunk-pair while the O matmul
  consumes the same fp8 A against bf16 V (shared rounding cancels in O/R)
- softmax reciprocal uses the fast approximate DVE op; phase-3 PSUM tiles
  are cast to bf16 on DVE/ACT alternately and DMA'd out as bf16 partials.
"""
import sys

sys.path.insert(0, '/opt/trn_rl_repo')

import numpy as np

B, S, HID = 2, 2048, 2048
H, D = 16, 128
NH = H // 4          # heads per core = 4
HC = HID // 128      # hid chunks = 16
TB = 512             # token block for projection
NTB = S // TB        # 4
QB = 512             # q block in attention
NQB = S // QB        # 4
NKCH = S // 128      # k chunks total = 16
SCALE = 1.0 / float(np.sqrt(D))
BASE = 10000.0
N_CORES = 8

_cache = {}


def _build():
    import concourse.bass as bass  # noqa: F401
    import concourse.tile as tile
    from concourse import bacc, mybir

    f32 = mybir.dt.float32
    bf16 = mybir.dt.bfloat16
    fp8 = mybir.dt.float8e4
    DR = mybir.MatmulPerfMode.DoubleRow
    EXP = mybir.ActivationFunctionType.Exp
    MULT = mybir.AluOpType.mult
    ADD = mybir.AluOpType.add
    # constant subtracted from scaled scores before exp so e^s fits fp8-e4m3
    # range (max causal score ~6.5 -> e^5.0 = 148 < 448); cancels in softmax
    CBIAS = -1.5

    nc = bacc.Bacc("TRN2", target_bir_lowering=False, debug=False,
                   num_devices=N_CORES)

    # x and the qkv weights arrive pre-tiled in their exact SBUF layouts so
    # every load is one contiguous large-line DMA
    xT = nc.dram_tensor("xT", [128, NTB * HC * TB], bf16,
                        kind="ExternalInput").ap()
    wqk = nc.dram_tensor("wqk", [128, 8 * HC * 128], bf16,
                         kind="ExternalInput").ap()
    wv = nc.dram_tensor("wv", [128, HC * NH * D], bf16,
                        kind="ExternalInput").ap()
    wo = nc.dram_tensor("wo", [NH * D, HID], bf16, kind="ExternalInput").ap()
    cosT = nc.dram_tensor("cosT", [D, S], bf16, kind="ExternalInput").ap()
    sinS = nc.dram_tensor("sinS", [D, S], bf16, kind="ExternalInput").ap()
    maskT = nc.dram_tensor("maskT", [128, 4 * QB], bf16, kind="ExternalInput").ap()
    ones_sq = nc.dram_tensor("ones_sq", [128, 128], bf16, kind="ExternalInput").ap()
    ones_8 = nc.dram_tensor("ones_8", [128, 256], fp8, kind="ExternalInput").ap()
    cbias = nc.dram_tensor("cbias", [128, 1], f32, kind="ExternalInput").ap()
    y = nc.dram_tensor("y", [S, HID], bf16, kind="ExternalOutput").ap()

    with tile.TileContext(nc) as tc:
        with tc.tile_pool(name="persist", bufs=1) as pp:
            # resident weights / tables (issued in dependency-urgency order:
            # wvt + first x block unblock the first matmul chain)
            wvt = pp.tile([128, HC * NH * D], bf16, tag="wvt")
            wqkt = pp.tile([128, 8 * HC * 128], bf16, tag="wqkt")
            cosF = pp.tile([D, S], bf16, tag="cosF")
            sinF = pp.tile([D, S], bf16, tag="sinF")
            tmask = pp.tile([128, 4 * QB], bf16, tag="tmask")
            t1s = pp.tile([128, 128], bf16, tag="t1s")
            t1s8 = pp.tile([128, 256], fp8, tag="t1s8")
            tcb = pp.tile([128, 1], f32, tag="tcb")
            wot = [pp.tile([128, HID], bf16, tag=f"wot{h}", name=f"wot{h}")
                   for h in range(NH)]

            # persistent activations
            qkT = [pp.tile([128, S], bf16, tag=f"qkT{i}", name=f"qkT{i}")
                   for i in range(8)]
            v_all = pp.tile([128, NKCH * NH * D], bf16, tag="v_all")
            outT = [pp.tile([128, S], bf16, tag=f"outT{h}", name=f"outT{h}")
                    for h in range(NH)]

            # ---- phase 1: V + Q/K projections, x streamed once ----
            with tc.tile_pool(name="p1x", bufs=2) as p1x, \
                 tc.tile_pool(name="rope", bufs=2) as rp, \
                 tc.tile_pool(name="psV", bufs=1, space="PSUM") as psV, \
                 tc.tile_pool(name="psQK", bufs=3, space="PSUM") as psQK:

                def load_xtb(jb):
                    # one large DMA: big transfers shard across all 16 DMA
                    # engines (~360GB/s); small per-chunk DMAs do not
                    xTb = p1x.tile([128, HC * TB], bf16, tag="xTb")
                    nc.sync.dma_start(
                        xTb[:], xT[:, jb * HC * TB:(jb + 1) * HC * TB])
                    return xTb

                # loads in strict consumption order; wvt and the first x
                # block are quarter-split and interleaved so the c-outer V
                # chain starts after ~512KB and then follows the DMA stream
                xTb0 = p1x.tile([128, HC * TB], bf16, tag="xTb")
                WQ = HC * NH * D // 4
                XQ = HC * TB // 4
                for q in range(4):
                    nc.sync.dma_start(wvt[:, q * WQ:(q + 1) * WQ],
                                      wv[:, q * WQ:(q + 1) * WQ])
                    nc.sync.dma_start(xTb0[:, q * XQ:(q + 1) * XQ],
                                      xT[:, q * XQ:(q + 1) * XQ])
                xtiles = [xTb0]
                nc.sync.dma_start(wqkt[:, :4 * HC * 128],
                                  wqk[:, :4 * HC * 128])
                nc.sync.dma_start(wqkt[:, 4 * HC * 128:],
                                  wqk[:, 4 * HC * 128:])
                nc.sync.dma_start(cosF[:, 0:TB], cosT[:, 0:TB])
                nc.sync.dma_start(sinF[:, 0:TB], sinS[:, 0:TB])
                xtiles.append(load_xtb(1))
                nc.sync.dma_start(cosF[:, TB:], cosT[:, TB:])
                nc.sync.dma_start(sinF[:, TB:], sinS[:, TB:])
                nc.sync.dma_start(tmask[:], maskT[:])
                nc.sync.dma_start(t1s[:], ones_sq[:])
                nc.sync.dma_start(t1s8[:], ones_8[:])
                nc.sync.dma_start(tcb[:], cbias[:])
                for h in range(NH):
                    nc.sync.dma_start(wot[h][:], wo[h * 128:(h + 1) * 128, :])

                for jb in range(NTB):
                    xTb = xtiles[jb]
                    # V projection (c-outer: follows the chunk DMA stream):
                    # 4 accumulators [128,512] in one 4-bank PSUM tile,
                    # evacuated with a single wide copy
                    Pv = psV.tile([128, 4 * NH * D], f32, tag="Pv")
                    for c in range(HC):
                        for t2 in range(TB // 128):
                            nc.tensor.matmul(
                                Pv[:, t2 * NH * D:(t2 + 1) * NH * D],
                                xTb[:, c * TB + t2 * 128:
                                    c * TB + (t2 + 1) * 128],
                                wvt[:, c * NH * D:(c + 1) * NH * D],
                                start=(c == 0), stop=(c == HC - 1))
                    nc.scalar.copy(
                        v_all[:, jb * 4 * NH * D:(jb + 1) * 4 * NH * D],
                        Pv[:])
                    # Q/K projection + RoPE
                    sl = slice(jb * TB, (jb + 1) * TB)
                    for cc in range(8):  # 4 q cols then 4 k cols
                        P = psQK.tile([128, TB], f32, tag="P")
                        for c in range(HC):
                            nc.tensor.matmul(
                                P[:],
                                wqkt[:, cc * HC * 128 + c * 128:
                                     cc * HC * 128 + (c + 1) * 128],
                                xTb[:, c * TB:(c + 1) * TB],
                                start=(c == 0), stop=(c == HC - 1))
                        u = rp.tile([128, TB], f32, tag="u")
                        nc.vector.tensor_copy(u[:], P[:])
                        rot = rp.tile([128, TB], f32, tag="rot")
                        nc.sync.dma_start(rot[0:64, :], u[64:128, :])
                        nc.sync.dma_start(rot[64:128, :], u[0:64, :])
                        m = rp.tile([128, TB], f32, tag="m")
                        nc.vector.tensor_tensor(
                            out=m[:], in0=rot[:], in1=sinF[:, sl], op=MULT)
                        t = rp.tile([128, TB], f32, tag="t")
                        nc.vector.tensor_tensor(
                            out=t[:], in0=u[:], in1=cosF[:, sl], op=MULT)
                        nc.vector.tensor_tensor(
                            out=qkT[cc][:, sl], in0=t[:], in1=m[:], op=ADD)
                    # prefetch x for jb+2 (after this jb's compute is issued
                    # so the buffer-rotation WAR dependency is correct)
                    if jb + 2 < NTB:
                        xtiles.append(load_xtb(jb + 2))

            # ---- phase 2: attention ----
            with tc.tile_pool(name="p2", bufs=4) as p2, \
                 tc.tile_pool(name="p2r", bufs=2) as p2r, \
                 tc.tile_pool(name="psS", bufs=2, space="PSUM") as psS, \
                 tc.tile_pool(name="psO", bufs=2, space="PSUM") as psO, \
                 tc.tile_pool(name="psR", bufs=2, space="PSUM") as psR:
                # globally software-pipelined (depth 2) across (jb4, h)
                # blocks: the O/R matmuls for pair k issue during pair k+2's
                # slot so the ACT exp (+ DVE mask) latency never stalls the
                # PE. Diagonal chunks at relative offset md only need
                # q >= 128*md — S/exp/mask/O/R all run on the narrowed range.
                def flush(p):
                    (kc0, A, O, R, nkc, h, qlos, last, qsl, is8) = p
                    for i in range(2):
                        kc = kc0 + i
                        ql = qlos[i]
                        nc.tensor.matmul(
                            O[:, ql:QB],
                            v_all[:, kc * NH * D + h * D:
                                  kc * NH * D + (h + 1) * D],
                            A[:, i * QB + ql:(i + 1) * QB],
                            start=(kc == 0), stop=(kc == nkc - 1),
                            skip_group_check=True)
                        if not is8:
                            nc.tensor.matmul(
                                R[:, ql:QB], t1s[:],
                                A[:, i * QB + ql:(i + 1) * QB],
                                start=(kc == 0), stop=(kc == nkc - 1),
                                skip_group_check=True)
                    if is8:
                        # one DoubleRow matmul sums both fp8 chunks at once
                        nc.tensor.matmul(
                            R[:],
                            t1s8[:].rearrange("p (two n) -> p two n", two=2),
                            A[:].rearrange("p (two q) -> p two q", two=2),
                            start=(kc0 == 0), stop=(kc0 + 1 == nkc - 1),
                            perf_mode=DR, skip_group_check=True)
                    if last:
                        rec = p2r.tile([128, QB], f32, tag="rec")
                        nc.vector.reciprocal_approx_fast(rec[:], R[:])
                        nc.vector.tensor_tensor(
                            out=outT[h][:, qsl], in0=O[:], in1=rec[:],
                            op=MULT)

                pending = []
                for jb4 in range(NQB):
                    qsl = slice(jb4 * QB, (jb4 + 1) * QB)
                    for h in range(NH):
                        qT_h, kT_h = qkT[h], qkT[NH + h]
                        O = psO.tile([128, QB], f32, tag="O")
                        R = psR.tile([128, QB], f32, tag="R")
                        nkc = (QB // 128) * (jb4 + 1)
                        for kp in range(nkc // 2):  # paired k-chunks
                            kc0 = 2 * kp
                            md = kc0 - (QB // 128) * jb4
                            # per-half diagonal offset and narrowed q range
                            mds = [kc0 + i - (QB // 128) * jb4
                                   for i in range(2)]
                            qlos = [max(0, 128 * m) for m in mds]
                            Sc = psS.tile([128, 2 * QB], f32, tag="S")
                            for i in range(2):
                                ql = qlos[i]
                                nc.tensor.matmul(
                                    Sc[:, i * QB + ql:(i + 1) * QB],
                                    kT_h[:, (kc0 + i) * 128:(kc0 + i + 1) * 128],
                                    qT_h[:, jb4 * QB + ql:(jb4 + 1) * QB],
                                    start=True, stop=True)
                            if md >= 0:  # diagonal pair: exp+mask per half
                                A = p2.tile([128, 2 * QB], bf16, tag="A",
                                            bufs=4)
                                Araw = p2.tile([128, 2 * QB], bf16, tag="Araw",
                                               bufs=4)
                                for i in range(2):
                                    ql = qlos[i]
                                    sl_i = slice(i * QB + ql, (i + 1) * QB)
                                    nc.scalar.activation(
                                        Araw[:, sl_i], Sc[:, sl_i], EXP,
                                        bias=tcb[:], scale=SCALE)
                                    nc.vector.tensor_tensor(
                                        out=A[:, sl_i], in0=Araw[:, sl_i],
                                        in1=tmask[:, mds[i] * QB + ql:
                                                  (mds[i] + 1) * QB],
                                        op=MULT)
                                is8 = False
                            else:
                                A = p2.tile([128, 2 * QB], fp8, tag="A8",
                                            bufs=4)
                                nc.scalar.activation(A[:], Sc[:], EXP,
                                                     bias=tcb[:], scale=SCALE)
                                is8 = True
                            if len(pending) >= 2:
                                flush(pending.pop(0))
                            pending.append((kc0, A, O, R, nkc, h, qlos,
                                            kp == nkc // 2 - 1, qsl, is8))
                for p in pending:
                    flush(p)

            # ---- phase 3: out projection (partial) ----
            with tc.tile_pool(name="p3", bufs=4) as p3, \
                 tc.tile_pool(name="ps3", bufs=4, space="PSUM") as ps3:
                for tch in range(S // 128):
                    for cb in range(HID // 512):
                        P3 = ps3.tile([128, 512], f32, tag="P3")
                        for h in range(NH):
                            nc.tensor.matmul(
                                P3[:],
                                outT[h][:, tch * 128:(tch + 1) * 128],
                                wot[h][:, cb * 512:(cb + 1) * 512],
                                start=(h == 0), stop=(h == NH - 1))
                        ys = p3.tile([128, 512], bf16, tag="ys")
                        if (tch * 4 + cb) % 2 == 0:
                            nc.vector.tensor_copy(ys[:], P3[:])
                        else:
                            nc.scalar.copy(ys[:], P3[:])
                        nc.sync.dma_start(
                            y[tch * 128:(tch + 1) * 128,
                              cb * 512:(cb + 1) * 512], ys[:])

    nc.compile()
    return nc


def _host_inputs(x, w_qkv, w_out):
    """Build the 8 per-core input maps."""
    import ml_dtypes
    bf16 = ml_dtypes.bfloat16

    # RoPE tables, transposed ([d, t]) with the rotate-half sign folded in.
    inv_freq = 1.0 / (BASE ** (np.arange(0, D, 2, dtype=np.float64) / D))
    pos = np.arange(S, dtype=np.float64)
    freqs = np.outer(inv_freq, pos)           # [64, S]
    cos_h = np.cos(freqs).astype(np.float32)
    sin_h = np.sin(freqs).astype(np.float32)
    cosT = np.concatenate([cos_h, cos_h], 0).astype(bf16)   # [128, S]
    sinS = np.concatenate([-sin_h, sin_h], 0).astype(bf16)  # signed sin

    # Causal masks for the 4 diagonal sub-blocks ([k-part, q-free])
    kp = np.arange(128)[:, None]
    qf = np.arange(QB)[None, :]
    maskT = np.concatenate(
        [(qf >= 128 * mm + kp).astype(bf16) for mm in range(4)], axis=1)

    w3 = np.asarray(w_qkv, np.float32).reshape(HID, 3, H, D)
    wo_full = np.asarray(w_out, np.float32).reshape(H, D, HID)
    x = np.asarray(x, np.float32)

    shared = {
        "cosT": cosT, "sinS": sinS, "maskT": maskT,
        "ones_sq": np.ones((128, 128), bf16),
        "ones_8": np.ones((128, 256), ml_dtypes.float8_e4m3fn),
        "cbias": np.full((128, 1), -1.5, np.float32),
    }
    # x pre-tiled to the SBUF layout: [p, jb, c, t]
    xt_b = []
    for b in range(B):
        xt = x[b].T.reshape(HC, 128, NTB, TB).transpose(1, 2, 0, 3)
        xt_b.append(np.ascontiguousarray(xt.reshape(128, -1)).astype(bf16))

    in_maps = []
    for c in range(N_CORES):
        b, hg = c // 4, c % 4
        heads = slice(4 * hg, 4 * hg + 4)
        wqk_c = w3[:, 0:2, heads, :].reshape(HID, 2 * NH * D)
        wqk_c = wqk_c.reshape(HC, 128, 8, 128).transpose(1, 2, 0, 3)
        wqk_c = np.ascontiguousarray(wqk_c.reshape(128, -1)).astype(bf16)
        wv_c = w3[:, 2, heads, :].reshape(HID, NH * D)
        wv_c = wv_c.reshape(HC, 128, NH * D).transpose(1, 0, 2)
        wv_c = np.ascontiguousarray(wv_c.reshape(128, -1)).astype(bf16)
        wo_c = np.ascontiguousarray(
            wo_full[heads].reshape(NH * D, HID)).astype(bf16)
        in_maps.append({
            "xT": xt_b[b], "wqk": wqk_c, "wv": wv_c, "wo": wo_c, **shared,
        })
    return in_maps


def kernel(x, w_qkv, w_out):
    from concourse.bass_utils import run_bass_kernel_spmd

    if "nc" not in _cache:
        _cache["nc"] = _build()
    nc = _cache["nc"]
    in_maps = _host_inputs(x, w_qkv, w_out)
    res = run_bass_kernel_spmd(nc, in_maps, core_ids=list(range(N_CORES)))
    out = np.zeros((B, S, HID), np.float32)
    for c in range(N_CORES):
        out[c // 4] += res.results[c]["y"].astype(np.float32)
    return out


# revision 45
# speedup vs baseline: 1.0042x; 1.0001x over previous
"""Multi-head causal attention (B=2, S=2048, HID=2048, H=16, D=128) on 8 TRN2
NeuronCores.

Sharding: core c handles batch b=c//4 and heads [4*(c%4) .. 4*(c%4)+3].
Each core computes qkv-projection + RoPE + causal attention + its partial
out-projection; the host sums the 4 partial outputs per batch (tensor-parallel
reduce) and stacks the 2 batches.

On-chip layout: all activations are kept transposed ([feature, token]) so the
whole chain runs on the PE array with no on-device transposes:
  qT/kT = W_qk^T-slice @ x^T   (RoPE applied during PSUM evacuation)
  S^T[k,q] = kT^T@qT ; A = exp(S^T*scale) (*causal mask)
  outT[d,q] = V^T-chunks @ A   (accumulated over k chunks)
  y[tok,col] = outT^T-chunks @ W_o-rows  (accumulated over heads)
Softmax row-sums come from a ones[128,128] matmul in the [k,q] layout (the
output is the denominator already broadcast across partitions); exp runs on
paired k-chunks ([128,1024] tiles) to amortize ACT overhead.

Performance structure (~330us on 8 cores, ~2.4GHz PE cadence):
- all matmul operands bf16 (1 cycle/row, half the DMA/SBUF of f32r); weights
  and x live in SBUF and are DMA'd exactly once as large contiguous
  transfers (big DMAs shard across all 16 engines; x/wvt are quarter-split
  and interleaved so the c-outer V chain starts after ~512KB)
- phase 2 is globally software-pipelined (depth 2) so ACT exp latency never
  stalls the PE; diagonal chunks are narrowed to q >= 128*md in S/exp/mask/
  O/R; off-diagonal pairs write exp output as fp8-e4m3 (scores get a -1.5
  bias pre-exp, cancelled by the softmax ratio, to fit e4m3 range) so the
  row-sum matmul runs one DoubleRow matmul per chunk-pair while the O matmul
  consumes the same fp8 A against bf16 V (shared rounding cancels in O/R)
- softmax reciprocal uses the fast approximate DVE op; phase-3 PSUM tiles
  are cast to bf16 on DVE/ACT alternately and DMA'd out as bf16 partials.
"""
import sys

sys.path.insert(0, '/opt/trn_rl_repo')

import numpy as np

B, S, HID = 2, 2048, 2048
H, D = 16, 128
NH = H // 4          # heads per core = 4
HC = HID // 128      # hid chunks = 16
TB = 512             # token block for projection
NTB = S // TB        # 4
QB = 512             # q block in attention
NQB = S // QB        # 4
NKCH = S // 128      # k chunks total = 16
SCALE = 1.0 / float(np.sqrt(D))
BASE = 10000.0
N_CORES = 8

_cache = {}


def _build():
    import concourse.bass as bass  # noqa: F401
    import concourse.tile as tile
    from concourse import bacc, mybir

    f32 = mybir.dt.float32
    bf16 = mybir.dt.bfloat16
    fp8 = mybir.dt.float8e4
    DR = mybir.MatmulPerfMode.DoubleRow
    EXP = mybir.ActivationFunctionType.Exp
    MULT = mybir.AluOpType.mult
    ADD = mybir.AluOpType.add
    # constant subtracted from scaled scores before exp so e^s fits fp8-e4m3
    # range (max causal score ~6.5 -> e^5.0 = 148 < 448); cancels in softmax
    CBIAS = -1.5

    nc = bacc.Bacc("TRN2", target_bir_lowering=False, debug=False,
                   num_devices=N_CORES)

    # x and the qkv weights arrive pre-tiled in their exact SBUF layouts so
    # every load is one contiguous large-line DMA
    xT = nc.dram_tensor("xT", [128, NTB * HC * TB], bf16,
                        kind="ExternalInput").ap()
    wqk = nc.dram_tensor("wqk", [128, 8 * HC * 128], bf16,
                         kind="ExternalInput").ap()
    wv = nc.dram_tensor("wv", [128, HC * NH * D], bf16,
                        kind="ExternalInput").ap()
    wo = nc.dram_tensor("wo", [NH * D, HID], bf16, kind="ExternalInput").ap()
    cosT = nc.dram_tensor("cosT", [D, S], bf16, kind="ExternalInput").ap()
    sinS = nc.dram_tensor("sinS", [D, S], bf16, kind="ExternalInput").ap()
    maskT = nc.dram_tensor("maskT", [128, 4 * QB], bf16, kind="ExternalInput").ap()
    ones_sq = nc.dram_tensor("ones_sq", [128, 128], bf16, kind="ExternalInput").ap()
    ones_8 = nc.dram_tensor("ones_8", [128, 256], fp8, kind="ExternalInput").ap()
    cbias = nc.dram_tensor("cbias", [128, 1], f32, kind="ExternalInput").ap()
    y = nc.dram_tensor("y", [S, HID], bf16, kind="ExternalOutput").ap()

    with tile.TileContext(nc) as tc:
        with tc.tile_pool(name="persist", bufs=1) as pp:
            # resident weights / tables (issued in dependency-urgency order:
            # wvt + first x block unblock the first matmul chain)
            wvt = pp.tile([128, HC * NH * D], bf16, tag="wvt")
            wqkt = pp.tile([128, 8 * HC * 128], bf16, tag="wqkt")
            cosF = pp.tile([D, S], bf16, tag="cosF")
            sinF = pp.tile([D, S], bf16, tag="sinF")
            tmask = pp.tile([128, 4 * QB], bf16, tag="tmask")
            t1s = pp.tile([128, 128], bf16, tag="t1s")
            t1s8 = pp.tile([128, 256], fp8, tag="t1s8")
            tcb = pp.tile([128, 1], f32, tag="tcb")
            wot = [pp.tile([128, HID], bf16, tag=f"wot{h}", name=f"wot{h}")
                   for h in range(NH)]

            # persistent activations
            qkT = [pp.tile([128, S], bf16, tag=f"qkT{i}", name=f"qkT{i}")
                   for i in range(8)]
            v_all = pp.tile([128, NKCH * NH * D], bf16, tag="v_all")
            outT = [pp.tile([128, S], bf16, tag=f"outT{h}", name=f"outT{h}")
                    for h in range(NH)]

            # ---- phase 1: V + Q/K projections, x streamed once ----
            with tc.tile_pool(name="p1x", bufs=2) as p1x, \
                 tc.tile_pool(name="rope", bufs=2) as rp, \
                 tc.tile_pool(name="psV", bufs=1, space="PSUM") as psV, \
                 tc.tile_pool(name="psQK", bufs=4, space="PSUM") as psQK:

                def load_xtb(jb):
                    # one large DMA: big transfers shard across all 16 DMA
                    # engines (~360GB/s); small per-chunk DMAs do not
                    xTb = p1x.tile([128, HC * TB], bf16, tag="xTb")
                    nc.sync.dma_start(
                        xTb[:], xT[:, jb * HC * TB:(jb + 1) * HC * TB])
                    return xTb

                # loads in strict consumption order; wvt and the first x
                # block are quarter-split and interleaved so the c-outer V
                # chain starts after ~512KB and then follows the DMA stream
                xTb0 = p1x.tile([128, HC * TB], bf16, tag="xTb")
                WQ = HC * NH * D // 8
                XQ = HC * TB // 8
                for q in range(8):
                    nc.sync.dma_start(wvt[:, q * WQ:(q + 1) * WQ],
                                      wv[:, q * WQ:(q + 1) * WQ])
                    nc.sync.dma_start(xTb0[:, q * XQ:(q + 1) * XQ],
                                      xT[:, q * XQ:(q + 1) * XQ])
                xtiles = [xTb0]
                nc.sync.dma_start(wqkt[:, :4 * HC * 128],
                                  wqk[:, :4 * HC * 128])
                nc.sync.dma_start(wqkt[:, 4 * HC * 128:],
                                  wqk[:, 4 * HC * 128:])
                nc.sync.dma_start(cosF[:, 0:TB], cosT[:, 0:TB])
                nc.sync.dma_start(sinF[:, 0:TB], sinS[:, 0:TB])
                xtiles.append(load_xtb(1))
                nc.sync.dma_start(cosF[:, TB:], cosT[:, TB:])
                nc.sync.dma_start(sinF[:, TB:], sinS[:, TB:])
                nc.sync.dma_start(tmask[:], maskT[:])
                nc.sync.dma_start(t1s[:], ones_sq[:])
                nc.sync.dma_start(t1s8[:], ones_8[:])
                nc.sync.dma_start(tcb[:], cbias[:])
                for h in range(NH):
                    nc.sync.dma_start(wot[h][:], wo[h * 128:(h + 1) * 128, :])

                for jb in range(NTB):
                    xTb = xtiles[jb]
                    # V projection (c-outer: follows the chunk DMA stream):
                    # 4 accumulators [128,512] in one 4-bank PSUM tile,
                    # evacuated with a single wide copy
                    Pv = psV.tile([128, 4 * NH * D], f32, tag="Pv")
                    for c in range(HC):
                        for t2 in range(TB // 128):
                            nc.tensor.matmul(
                                Pv[:, t2 * NH * D:(t2 + 1) * NH * D],
                                xTb[:, c * TB + t2 * 128:
                                    c * TB + (t2 + 1) * 128],
                                wvt[:, c * NH * D:(c + 1) * NH * D],
                                start=(c == 0), stop=(c == HC - 1))
                    nc.scalar.copy(
                        v_all[:, jb * 4 * NH * D:(jb + 1) * 4 * NH * D],
                        Pv[:])
                    # Q/K projection + RoPE
                    sl = slice(jb * TB, (jb + 1) * TB)
                    for cc in range(8):  # 4 q cols then 4 k cols
                        P = psQK.tile([128, TB], f32, tag="P")
                        for c in range(HC):
                            nc.tensor.matmul(
                                P[:],
                                wqkt[:, cc * HC * 128 + c * 128:
                                     cc * HC * 128 + (c + 1) * 128],
                                xTb[:, c * TB:(c + 1) * TB],
                                start=(c == 0), stop=(c == HC - 1))
                        u = rp.tile([128, TB], f32, tag="u")
                        nc.vector.tensor_copy(u[:], P[:])
                        rot = rp.tile([128, TB], f32, tag="rot")
                        nc.sync.dma_start(rot[0:64, :], u[64:128, :])
                        nc.sync.dma_start(rot[64:128, :], u[0:64, :])
                        m = rp.tile([128, TB], f32, tag="m")
                        nc.vector.tensor_tensor(
                            out=m[:], in0=rot[:], in1=sinF[:, sl], op=MULT)
                        t = rp.tile([128, TB], f32, tag="t")
                        nc.vector.tensor_tensor(
                            out=t[:], in0=u[:], in1=cosF[:, sl], op=MULT)
                        nc.vector.tensor_tensor(
                            out=qkT[cc][:, sl], in0=t[:], in1=m[:], op=ADD)
                    # prefetch x for jb+2 (after this jb's compute is issued
                    # so the buffer-rotation WAR dependency is correct)
                    if jb + 2 < NTB:
                        xtiles.append(load_xtb(jb + 2))

            # ---- phase 2: attention ----
            with tc.tile_pool(name="p2", bufs=4) as p2, \
                 tc.tile_pool(name="p2r", bufs=2) as p2r, \
                 tc.tile_pool(name="psS", bufs=2, space="PSUM") as psS, \
                 tc.tile_pool(name="psO", bufs=2, space="PSUM") as psO, \
                 tc.tile_pool(name="psR", bufs=2, space="PSUM") as psR:
                # globally software-pipelined (depth 2) across (jb4, h)
                # blocks: the O/R matmuls for pair k issue during pair k+2's
                # slot so the ACT exp (+ DVE mask) latency never stalls the
                # PE. Diagonal chunks at relative offset md only need
                # q >= 128*md — S/exp/mask/O/R all run on the narrowed range.
                def flush(p):
                    (kc0, A, O, R, nkc, h, qlos, last, qsl, is8) = p
                    for i in range(2):
                        kc = kc0 + i
                        ql = qlos[i]
                        nc.tensor.matmul(
                            O[:, ql:QB],
                            v_all[:, kc * NH * D + h * D:
                                  kc * NH * D + (h + 1) * D],
                            A[:, i * QB + ql:(i + 1) * QB],
                            start=(kc == 0), stop=(kc == nkc - 1),
                            skip_group_check=True)
                        if not is8:
                            nc.tensor.matmul(
                                R[:, ql:QB], t1s[:],
                                A[:, i * QB + ql:(i + 1) * QB],
                                start=(kc == 0), stop=(kc == nkc - 1),
                                skip_group_check=True)
                    if is8:
                        # one DoubleRow matmul sums both fp8 chunks at once
                        nc.tensor.matmul(
                            R[:],
                            t1s8[:].rearrange("p (two n) -> p two n", two=2),
                            A[:].rearrange("p (two q) -> p two q", two=2),
                            start=(kc0 == 0), stop=(kc0 + 1 == nkc - 1),
                            perf_mode=DR, skip_group_check=True)
                    if last:
                        rec = p2r.tile([128, QB], f32, tag="rec")
                        nc.vector.reciprocal_approx_fast(rec[:], R[:])
                        nc.vector.tensor_tensor(
                            out=outT[h][:, qsl], in0=O[:], in1=rec[:],
                            op=MULT)

                pending = []
                for jb4 in range(NQB):
                    qsl = slice(jb4 * QB, (jb4 + 1) * QB)
                    for h in range(NH):
                        qT_h, kT_h = qkT[h], qkT[NH + h]
                        O = psO.tile([128, QB], f32, tag="O")
                        R = psR.tile([128, QB], f32, tag="R")
                        nkc = (QB // 128) * (jb4 + 1)
                        for kp in range(nkc // 2):  # paired k-chunks
                            kc0 = 2 * kp
                            md = kc0 - (QB // 128) * jb4
                            # per-half diagonal offset and narrowed q range
                            mds = [kc0 + i - (QB // 128) * jb4
                                   for i in range(2)]
                            qlos = [max(0, 128 * m) for m in mds]
                            Sc = psS.tile([128, 2 * QB], f32, tag="S")
                            for i in range(2):
                                ql = qlos[i]
                                nc.tensor.matmul(
                                    Sc[:, i * QB + ql:(i + 1) * QB],
                                    kT_h[:, (kc0 + i) * 128:(kc0 + i + 1) * 128],
                                    qT_h[:, jb4 * QB + ql:(jb4 + 1) * QB],
                                    start=True, stop=True)
                            if md >= 0:  # diagonal pair: exp+mask per half
                                A = p2.tile([128, 2 * QB], bf16, tag="A",
                                            bufs=4)
                                Araw = p2.tile([128, 2 * QB], bf16, tag="Araw",
                                               bufs=4)
                                for i in range(2):
                                    ql = qlos[i]
                                    sl_i = slice(i * QB + ql, (i + 1) * QB)
                                    nc.scalar.activation(
                                        Araw[:, sl_i], Sc[:, sl_i], EXP,
                                        bias=tcb[:], scale=SCALE)
                                    nc.vector.tensor_tensor(
                                        out=A[:, sl_i], in0=Araw[:, sl_i],
                                        in1=tmask[:, mds[i] * QB + ql:
                                                  (mds[i] + 1) * QB],
                                        op=MULT)
                                is8 = False
                            else:
                                A = p2.tile([128, 2 * QB], fp8, tag="A8",
                                            bufs=4)
                                nc.scalar.activation(A[:], Sc[:], EXP,
                                                     bias=tcb[:], scale=SCALE)
                                is8 = True
                            if len(pending) >= 2:
                                flush(pending.pop(0))
                            pending.append((kc0, A, O, R, nkc, h, qlos,
                                            kp == nkc // 2 - 1, qsl, is8))
                for p in pending:
                    flush(p)

            # ---- phase 3: out projection (partial) ----
            with tc.tile_pool(name="p3", bufs=4) as p3, \
                 tc.tile_pool(name="ps3", bufs=4, space="PSUM") as ps3:
                for tch in range(S // 128):
                    for cb in range(HID // 512):
                        P3 = ps3.tile([128, 512], f32, tag="P3")
                        for h in range(NH):
                            nc.tensor.matmul(
                                P3[:],
                                outT[h][:, tch * 128:(tch + 1) * 128],
                                wot[h][:, cb * 512:(cb + 1) * 512],
                                start=(h == 0), stop=(h == NH - 1))
                        ys = p3.tile([128, 512], bf16, tag="ys")
                        if (tch * 4 + cb) % 2 == 0:
                            nc.vector.tensor_copy(ys[:], P3[:])
                        else:
                            nc.scalar.copy(ys[:], P3[:])
                        nc.sync.dma_start(
                            y[tch * 128:(tch + 1) * 128,
                              cb * 512:(cb + 1) * 512], ys[:])

    nc.compile()
    return nc


def _host_inputs(x, w_qkv, w_out):
    """Build the 8 per-core input maps."""
    import ml_dtypes
    bf16 = ml_dtypes.bfloat16

    # RoPE tables, transposed ([d, t]) with the rotate-half sign folded in.
    inv_freq = 1.0 / (BASE ** (np.arange(0, D, 2, dtype=np.float64) / D))
    pos = np.arange(S, dtype=np.float64)
    freqs = np.outer(inv_freq, pos)           # [64, S]
    cos_h = np.cos(freqs).astype(np.float32)
    sin_h = np.sin(freqs).astype(np.float32)
    cosT = np.concatenate([cos_h, cos_h], 0).astype(bf16)   # [128, S]
    sinS = np.concatenate([-sin_h, sin_h], 0).astype(bf16)  # signed sin

    # Causal masks for the 4 diagonal sub-blocks ([k-part, q-free])
    kp = np.arange(128)[:, None]
    qf = np.arange(QB)[None, :]
    maskT = np.concatenate(
        [(qf >= 128 * mm + kp).astype(bf16) for mm in range(4)], axis=1)

    w3 = np.asarray(w_qkv, np.float32).reshape(HID, 3, H, D)
    wo_full = np.asarray(w_out, np.float32).reshape(H, D, HID)
    x = np.asarray(x, np.float32)

    shared = {
        "cosT": cosT, "sinS": sinS, "maskT": maskT,
        "ones_sq": np.ones((128, 128), bf16),
        "ones_8": np.ones((128, 256), ml_dtypes.float8_e4m3fn),
        "cbias": np.full((128, 1), -1.5, np.float32),
    }
    # x pre-tiled to the SBUF layout: [p, jb, c, t]
    xt_b = []
    for b in range(B):
        xt = x[b].T.reshape(HC, 128, NTB, TB).transpose(1, 2, 0, 3)
        xt_b.append(np.ascontiguousarray(xt.reshape(128, -1)).astype(bf16))

    in_maps = []
    for c in range(N_CORES):
        b, hg = c // 4, c % 4
        heads = slice(4 * hg, 4 * hg + 4)
        wqk_c = w3[:, 0:2, heads, :].reshape(HID, 2 * NH * D)
        wqk_c = wqk_c.reshape(HC, 128, 8, 128).transpose(1, 2, 0, 3)
        wqk_c = np.ascontiguousarray(wqk_c.reshape(128, -1)).astype(bf16)
        wv_c = w3[:, 2, heads, :].reshape(HID, NH * D)
        wv_c = wv_c.reshape(HC, 128, NH * D).transpose(1, 0, 2)
        wv_c = np.ascontiguousarray(wv_c.reshape(128, -1)).astype(bf16)
        wo_c = np.ascontiguousarray(
            wo_full[heads].reshape(NH * D, HID)).astype(bf16)
        in_maps.append({
            "xT": xt_b[b], "wqk": wqk_c, "wv": wv_c, "wo": wo_c, **shared,
        })
    return in_maps


def kernel(x, w_qkv, w_out):
    from concourse.bass_utils import run_bass_kernel_spmd

    if "nc" not in _cache:
        _cache["nc"] = _build()
    nc = _cache["nc"]
    in_maps = _host_inputs(x, w_qkv, w_out)
    res = run_bass_kernel_spmd(nc, in_maps, core_ids=list(range(N_CORES)))
    out = np.zeros((B, S, HID), np.float32)
    for c in range(N_CORES):
        out[c // 4] += res.results[c]["y"].astype(np.float32)
    return out
